# revision 6
# baseline (speedup 1.0000x reference)
"""Multi-head attention (B=4, H=16, S=2048, Dqk=Dv=64, HID=1024) on 8 trn2 cores.

Returns (out, attn) matching the reference:
    scores = q @ k^T + mask; attn = softmax(scores); ctx = attn @ v
    out = rearrange(ctx, 'b h s d -> b s (h d)') @ W_proj^T

Sharding: the 64 (b, h) pairs are split 8-per-core (head/data parallel).
Each core computes its pairs' attention probabilities (stored transposed,
[kj, qi], so that both matmuls and the final projection consume natural
layouts) plus a partial projection over its 8 heads; the host sums the
two per-batch partials (the unshard step for the tensor-parallel output).

Device-side layout notes:
  - k^T / q^T are packed per pair into one [64, 4096] block -> single DMA.
  - v gets a ones column appended ([*, 65]); the attn@v matmul then yields
    the softmax denominators in row 64 of PSUM for free.
  - Softmax skips the max-subtraction: |scores| < ~60 for any plausible
    input here, comfortably inside fp32 exp range; -inf masks still work
    (exp(-inf) = 0).
  - matmuls run in fp32r (1 cycle/row at N=512) -> tf32-like rounding,
    ~2e-4 relative error on scores.
"""

import contextlib

import numpy as np

import concourse.bass as bass
import concourse.bacc as bacc
import concourse.mybir as mybir
import concourse.tile as tile
from concourse.bass_utils import run_bass_kernel_spmd

F32 = mybir.dt.float32
F32R = mybir.dt.float32r
EXP = mybir.ActivationFunctionType.Exp
COPY = mybir.ActivationFunctionType.Copy

B, H, S, D, HID = 4, 16, 2048, 64, 1024
NCORES = 8
PPC = B * H // NCORES       # pairs per core = 8
QCHUNK = 512                # query-chunk width (matmul N)
NQC = S // QCHUNK           # 4
NT = S // 128               # 16 key blocks
TH = 2                      # e-tile groups per chunk (8 key blocks each)

_cache: dict = {}
_last_in_maps: list | None = None


def _build(with_mask: bool):
    nc = bacc.Bacc()
    kq_d = nc.declare_dram_parameter("kq", [PPC, D, 2 * S], F32, isOutput=False)
    v_d = nc.declare_dram_parameter("vS", [PPC, 128, NT, 65], F32, isOutput=False)
    wp_d = nc.declare_dram_parameter("wp", [PPC * D, HID], F32, isOutput=False)
    if with_mask:
        m_d = nc.declare_dram_parameter("mT", [PPC, S, S], F32, isOutput=False)
    attn_d = nc.declare_dram_parameter("attn_t", [PPC, S, S], F32, isOutput=True)
    y_d = nc.declare_dram_parameter("y", [S, HID], F32, isOutput=True)

    with tile.TileContext(nc) as tc:
        with (
            tc.tile_pool(name="kq_p", bufs=2) as kq_p,
            tc.tile_pool(name="v_p", bufs=2) as v_p,
            tc.tile_pool(name="e_p", bufs=3 if with_mask else 4) as e_p,
            tc.tile_pool(name="rb_p", bufs=3) as rb_p,
            tc.tile_pool(name="rc_p", bufs=3) as rc_p,
            tc.tile_pool(name="xt_p", bufs=1) as xt_p,
            tc.tile_pool(name="wp_p", bufs=1) as wp_p,
            tc.tile_pool(name="yo_p", bufs=3) as yo_p,
            tc.tile_pool(name="one_p", bufs=1) as one_p,
            tc.tile_pool(name="ps_p", bufs=3, space="PSUM") as ps_p,
            tc.tile_pool(name="po_p", bufs=2, space="PSUM") as po_p,
            tc.tile_pool(name="pb_p", bufs=2, space="PSUM") as pb_p,
            (
                tc.tile_pool(name="m_p", bufs=2)
                if with_mask
                else contextlib.nullcontext()
            ) as m_p,
        ):
            ones_t = one_p.tile([1, 128], F32, name="ones")
            nc.vector.memset(ones_t, 1.0)

            wp_t = []
            for i in range(4):
                t = wp_p.tile([128, HID], F32R, name=f"wp{i}")
                nc.sync.dma_start(
                    out=t, in_=wp_d[i * 128 : (i + 1) * 128, :].bitcast(F32R)
                )
                wp_t.append(t)

            xT = [xt_p.tile([128, S], F32R, name=f"xT{i}") for i in range(4)]

            for p in range(PPC):
                kq_t = kq_p.tile([D, 2 * S], F32R, name="kq_t")
                nc.sync.dma_start(out=kq_t, in_=kq_d[p].bitcast(F32R))
                v_t = v_p.tile([128, NT, 65], F32R, name="v_t")
                nc.sync.dma_start(out=v_t, in_=v_d[p].bitcast(F32R))

                for qc in range(NQC):
                    q_sl = kq_t[:, S + qc * QCHUNK : S + (qc + 1) * QCHUNK]
                    if with_mask:
                        m_big = []
                        for th in range(TH):
                            mb = m_p.tile([128, 8, QCHUNK], F32, name="m_big")
                            nc.sync.dma_start(
                                out=mb,
                                in_=m_d[
                                    p,
                                    th * 1024 : (th + 1) * 1024,
                                    qc * QCHUNK : (qc + 1) * QCHUNK,
                                ].rearrange("(ts pp) j -> pp ts j", pp=128),
                            )
                            m_big.append(mb)
                    e_big = [
                        e_p.tile([128, 8, QCHUNK], F32R, name="e_big")
                        for _ in range(TH)
                    ]
                    ps_o = po_p.tile([65, QCHUNK], F32, name="ps_o")
                    for t in range(NT):
                        th, ts = divmod(t, 8)
                        ps_s = ps_p.tile([128, QCHUNK], F32, name="ps_s")
                        nc.tensor.matmul(
                            ps_s[:],
                            kq_t[:, t * 128 : (t + 1) * 128],
                            q_sl,
                            start=True,
                            stop=True,
                        )
                        if with_mask:
                            nc.vector.tensor_add(
                                ps_s[:], ps_s[:], m_big[th][:, ts, :]
                            )
                        nc.scalar.activation(e_big[th][:, ts, :], ps_s[:], EXP)
                        nc.tensor.matmul(
                            ps_o[:],
                            v_t[:, t, :],
                            e_big[th][:, ts, :],
                            start=(t == 0),
                            stop=(t == NT - 1),
                        )
                    # reciprocal of the softmax sums (row 64), broadcast to
                    # all 128 partitions via a K=1 fp32 matmul
                    rc = rc_p.tile([1, QCHUNK], F32, name="rc")
                    nc.vector.reciprocal(rc[:], ps_o[64:65, :])
                    ps_b = pb_p.tile([128, QCHUNK], F32, name="ps_b")
                    nc.tensor.matmul(ps_b[:], ones_t[:], rc[:], start=True, stop=True)
                    rb = rb_p.tile([128, QCHUNK], F32, name="rb")
                    nc.scalar.activation(rb[:], ps_b[:], COPY)

                    # normalized context (transposed) into the projection stack
                    xi, xr = divmod(p, 2)
                    nc.vector.tensor_mul(
                        xT[xi][xr * 64 : (xr + 1) * 64, qc * QCHUNK : (qc + 1) * QCHUNK],
                        ps_o[0:64, :],
                        rb[0:64, :],
                    )
                    # normalize attention probs in place and store
                    for th in range(TH):
                        for ts in range(8):
                            nc.vector.tensor_mul(
                                e_big[th][:, ts, :], e_big[th][:, ts, :], rb[:]
                            )
                        out_ap = attn_d[
                            p,
                            th * 1024 : (th + 1) * 1024,
                            qc * QCHUNK : (qc + 1) * QCHUNK,
                        ].rearrange("(ts pp) j -> pp ts j", pp=128)
                        nc.sync.dma_start(out=out_ap.bitcast(F32R), in_=e_big[th][:])

            # final projection: y[s, :] = sum_i xT[i][:, s]^T @ wp[i]
            for sc in range(NT):
                y_sb = yo_p.tile([128, HID], F32, name="y_sb")
                for hc in range(2):
                    ps_y = ps_p.tile([128, QCHUNK], F32, name="ps_s")
                    for i in range(4):
                        nc.tensor.matmul(
                            ps_y[:],
                            xT[i][:, sc * 128 : (sc + 1) * 128],
                            wp_t[i][:, hc * QCHUNK : (hc + 1) * QCHUNK],
                            start=(i == 0),
                            stop=(i == 3),
                        )
                    nc.scalar.activation(
                        y_sb[:, hc * QCHUNK : (hc + 1) * QCHUNK], ps_y[:], COPY
                    )
                nc.sync.dma_start(
                    out=y_d[sc * 128 : (sc + 1) * 128, :], in_=y_sb[:]
                )
    nc.compile()
    return nc


def _get_nc(with_mask: bool):
    if with_mask not in _cache:
        _cache[with_mask] = _build(with_mask)
    return _cache[with_mask]


def kernel(q, k, v, attention_mask, W_proj):
    q = np.asarray(q, dtype=np.float32)
    k = np.asarray(k, dtype=np.float32)
    v = np.asarray(v, dtype=np.float32)
    attention_mask = np.asarray(attention_mask, dtype=np.float32)
    W_proj = np.asarray(W_proj, dtype=np.float32)

    with_mask = bool(np.any(attention_mask))

    # --- host-side layout marshalling (sharding) ---
    k_t = np.ascontiguousarray(k.transpose(0, 1, 3, 2)).reshape(B * H, D, S)
    q_t = np.ascontiguousarray(q.transpose(0, 1, 3, 2)).reshape(B * H, D, S)
    kq = np.concatenate([k_t, q_t], axis=2)  # [64, 64, 4096]
    v_flat = v.reshape(B * H, S, D)
    v_aug = np.concatenate(
        [v_flat, np.ones((B * H, S, 1), np.float32)], axis=2
    )  # [64, 2048, 65]
    # partition-major swizzle for efficient DMA: [64, 128, 16, 65]
    v_sw = np.ascontiguousarray(v_aug.reshape(B * H, NT, 128, 65).transpose(0, 2, 1, 3))
    wp_full = np.ascontiguousarray(W_proj.T)  # [H*Dv, HID]
    if with_mask:
        m_t = np.ascontiguousarray(attention_mask.transpose(0, 1, 3, 2)).reshape(
            B * H, S, S
        )

    in_maps = []
    for c in range(NCORES):
        sl = slice(c * PPC, (c + 1) * PPC)
        m = {
            "kq": kq[sl],
            "vS": v_sw[sl],
            "wp": wp_full[(c % 2) * 512 : (c % 2) * 512 + 512],
        }
        if with_mask:
            m["mT"] = m_t[sl]
        in_maps.append(m)

    global _last_in_maps
    _last_in_maps = in_maps

    nc = _get_nc(with_mask)
    res = run_bass_kernel_spmd(nc, in_maps, core_ids=list(range(NCORES))).results

    # --- gather / unshard ---
    attn_t = np.concatenate([r["attn_t"] for r in res], axis=0)  # [64, S(kj), S(qi)]
    attn = attn_t.reshape(B, H, S, S).transpose(0, 1, 3, 2)
    out = np.stack([res[2 * b]["y"] + res[2 * b + 1]["y"] for b in range(B)])
    return (out, attn)


# revision 7
# speedup vs baseline: 1.0936x; 1.0936x over previous
"""Multi-head attention (B=4, H=16, S=2048, Dqk=Dv=64, HID=1024) on 8 trn2 cores.

Returns (out, attn) matching the reference:
    scores = q @ k^T + mask; attn = softmax(scores); ctx = attn @ v
    out = rearrange(ctx, 'b h s d -> b s (h d)') @ W_proj^T

Sharding: the 64 (b, h) pairs are split 8-per-core (head/data parallel).
Each core computes its pairs' attention probabilities (stored transposed,
[kj, qi], so that both matmuls and the final projection consume natural
layouts) plus a partial projection over its 8 heads; the host sums the
two per-batch partials (the unshard step for the tensor-parallel output).

Device-side layout notes:
  - k^T / q^T are packed per pair into one [64, 4096] block -> single DMA.
  - v gets a ones column appended ([*, 65]); the attn@v matmul then yields
    the softmax denominators in row 64 of PSUM for free.
  - Softmax skips the max-subtraction: |scores| < ~60 for any plausible
    input here, comfortably inside fp32 exp range; -inf masks still work
    (exp(-inf) = 0).
  - matmuls run in fp32r (1 cycle/row at N=512) -> tf32-like rounding,
    ~2e-4 relative error on scores.
"""

import contextlib

import numpy as np

import concourse.bass as bass
import concourse.bacc as bacc
import concourse.mybir as mybir
import concourse.tile as tile
from concourse.bass_utils import run_bass_kernel_spmd

F32 = mybir.dt.float32
F32R = mybir.dt.float32r
EXP = mybir.ActivationFunctionType.Exp
COPY = mybir.ActivationFunctionType.Copy

B, H, S, D, HID = 4, 16, 2048, 64, 1024
NCORES = 8
PPC = B * H // NCORES       # pairs per core = 8
QCHUNK = 512                # query-chunk width (matmul N)
NQC = S // QCHUNK           # 4
NT = S // 128               # 16 key blocks
TH = 2                      # e-tile groups per chunk (8 key blocks each)

_cache: dict = {}
_last_in_maps: list | None = None


def _build(with_mask: bool):
    nc = bacc.Bacc()
    kq_d = nc.declare_dram_parameter("kq", [PPC, 2 * D, 2 * S], F32, isOutput=False)
    v_d = nc.declare_dram_parameter("vS", [PPC, 128, NT, 65], F32, isOutput=False)
    wp_d = nc.declare_dram_parameter("wp", [PPC * D, HID], F32, isOutput=False)
    if with_mask:
        m_d = nc.declare_dram_parameter("mT", [PPC, S, S], F32, isOutput=False)
    attn_d = nc.declare_dram_parameter("attn_t", [PPC, S, S], F32, isOutput=True)
    y_d = nc.declare_dram_parameter("y", [S, HID], F32, isOutput=True)

    with tile.TileContext(nc) as tc:
        with (
            tc.tile_pool(name="kq_p", bufs=2) as kq_p,
            tc.tile_pool(name="v_p", bufs=2) as v_p,
            tc.tile_pool(name="e_p", bufs=3 if with_mask else 4) as e_p,
            tc.tile_pool(name="rb_p", bufs=3) as rb_p,
            tc.tile_pool(name="rc_p", bufs=3) as rc_p,
            tc.tile_pool(name="xt_p", bufs=1) as xt_p,
            tc.tile_pool(name="wp_p", bufs=1) as wp_p,
            tc.tile_pool(name="yo_p", bufs=3) as yo_p,
            tc.tile_pool(name="one_p", bufs=1) as one_p,
            tc.tile_pool(name="ps_p", bufs=4, space="PSUM") as ps_p,
            tc.tile_pool(name="po_p", bufs=2, space="PSUM") as po_p,
            tc.tile_pool(name="pb_p", bufs=2, space="PSUM") as pb_p,
            (
                tc.tile_pool(name="m_p", bufs=2)
                if with_mask
                else contextlib.nullcontext()
            ) as m_p,
        ):
            ones_t = one_p.tile([1, 128], F32, name="ones")
            nc.vector.memset(ones_t, 1.0)

            wp_t = []
            for i in range(4):
                t = wp_p.tile([128, HID], F32R, name=f"wp{i}")
                nc.sync.dma_start(
                    out=t, in_=wp_d[i * 128 : (i + 1) * 128, :].bitcast(F32R)
                )
                wp_t.append(t)

            xT = [xt_p.tile([128, S], F32R, name=f"xT{i}") for i in range(4)]

            for p in range(PPC):
                kq_t = kq_p.tile([2 * D, 2 * S], F32R, name="kq_t")
                nc.sync.dma_start(out=kq_t, in_=kq_d[p].bitcast(F32R))
                v_t = v_p.tile([128, NT, 65], F32R, name="v_t")
                nc.sync.dma_start(out=v_t, in_=v_d[p].bitcast(F32R))

                for qc in range(NQC):
                    q_lo = kq_t[0:D, S + qc * QCHUNK : S + (qc + 1) * QCHUNK]
                    q_hi = kq_t[D:2 * D, S + qc * QCHUNK : S + (qc + 1) * QCHUNK]
                    if with_mask:
                        m_big = []
                        for th in range(TH):
                            mb = m_p.tile([128, 8, QCHUNK], F32, name="m_big")
                            nc.sync.dma_start(
                                out=mb,
                                in_=m_d[
                                    p,
                                    th * 1024 : (th + 1) * 1024,
                                    qc * QCHUNK : (qc + 1) * QCHUNK,
                                ].rearrange("(ts pp) j -> pp ts j", pp=128),
                            )
                            m_big.append(mb)
                    e_big = [
                        e_p.tile([128, 8, QCHUNK], F32R, name="e_big")
                        for _ in range(TH)
                    ]
                    ps_o = po_p.tile([65, QCHUNK], F32, name="ps_o")
                    for t in range(NT):
                        th, ts = divmod(t, 8)
                        ps_s = ps_p.tile([128, QCHUNK], F32, name="ps_s")
                        if t % 2 == 0:
                            nc.tensor.matmul(
                                ps_s[:],
                                kq_t[0:D, t * 128 : (t + 1) * 128],
                                q_lo,
                                start=True,
                                stop=True,
                                tile_position=(0, 0),
                            )
                        else:
                            nc.tensor.matmul(
                                ps_s[:],
                                kq_t[D : 2 * D, t * 128 : (t + 1) * 128],
                                q_hi,
                                start=True,
                                stop=True,
                                tile_position=(64, 0),
                            )
                        if with_mask:
                            nc.vector.tensor_add(
                                ps_s[:], ps_s[:], m_big[th][:, ts, :]
                            )
                        nc.scalar.activation(e_big[th][:, ts, :], ps_s[:], EXP)
                        nc.tensor.matmul(
                            ps_o[:],
                            v_t[:, t, :],
                            e_big[th][:, ts, :],
                            start=(t == 0),
                            stop=(t == NT - 1),
                        )
                    # reciprocal of the softmax sums (row 64), broadcast to
                    # all 128 partitions via a K=1 fp32 matmul
                    rc = rc_p.tile([1, QCHUNK], F32, name="rc")
                    rc_s = rc_p.tile([1, QCHUNK], F32, name="rc_s")
                    nc.vector.tensor_copy(rc_s[:], ps_o[64:65, :])
                    nc.vector.reciprocal_approx_fast(rc[:], rc_s[:])
                    ps_b = pb_p.tile([128, QCHUNK], F32, name="ps_b")
                    nc.tensor.matmul(ps_b[:], ones_t[:], rc[:], start=True, stop=True)
                    rb = rb_p.tile([128, QCHUNK], F32, name="rb")
                    nc.scalar.activation(rb[:], ps_b[:], COPY)

                    # normalized context (transposed) into the projection stack
                    xi, xr = divmod(p, 2)
                    nc.vector.tensor_mul(
                        xT[xi][xr * 64 : (xr + 1) * 64, qc * QCHUNK : (qc + 1) * QCHUNK],
                        ps_o[0:64, :],
                        rb[0:64, :],
                    )
                    # normalize attention probs in place and store
                    for th in range(TH):
                        for ts in range(8):
                            nc.vector.tensor_mul(
                                e_big[th][:, ts, :], e_big[th][:, ts, :], rb[:]
                            )
                        out_ap = attn_d[
                            p,
                            th * 1024 : (th + 1) * 1024,
                            qc * QCHUNK : (qc + 1) * QCHUNK,
                        ].rearrange("(ts pp) j -> pp ts j", pp=128)
                        nc.sync.dma_start(out=out_ap.bitcast(F32R), in_=e_big[th][:])

            # final projection: y[s, :] = sum_i xT[i][:, s]^T @ wp[i]
            for sc in range(NT):
                y_sb = yo_p.tile([128, HID], F32, name="y_sb")
                for hc in range(2):
                    ps_y = ps_p.tile([128, QCHUNK], F32, name="ps_s")
                    for i in range(4):
                        nc.tensor.matmul(
                            ps_y[:],
                            xT[i][:, sc * 128 : (sc + 1) * 128],
                            wp_t[i][:, hc * QCHUNK : (hc + 1) * QCHUNK],
                            start=(i == 0),
                            stop=(i == 3),
                        )
                    nc.scalar.activation(
                        y_sb[:, hc * QCHUNK : (hc + 1) * QCHUNK], ps_y[:], COPY
                    )
                nc.sync.dma_start(
                    out=y_d[sc * 128 : (sc + 1) * 128, :], in_=y_sb[:]
                )
    nc.compile()
    return nc


def _get_nc(with_mask: bool):
    if with_mask not in _cache:
        _cache[with_mask] = _build(with_mask)
    return _cache[with_mask]


def kernel(q, k, v, attention_mask, W_proj):
    q = np.asarray(q, dtype=np.float32)
    k = np.asarray(k, dtype=np.float32)
    v = np.asarray(v, dtype=np.float32)
    attention_mask = np.asarray(attention_mask, dtype=np.float32)
    W_proj = np.asarray(W_proj, dtype=np.float32)

    with_mask = bool(np.any(attention_mask))

    # --- host-side layout marshalling (sharding) ---
    k_t = np.ascontiguousarray(k.transpose(0, 1, 3, 2)).reshape(B * H, D, S)
    q_t = np.ascontiguousarray(q.transpose(0, 1, 3, 2)).reshape(B * H, D, S)
    kq1 = np.concatenate([k_t, q_t], axis=2)  # [64, 64, 4096]
    kq = np.concatenate([kq1, kq1], axis=1)  # [64, 128, 4096] duplicated halves
    v_flat = v.reshape(B * H, S, D)
    v_aug = np.concatenate(
        [v_flat, np.ones((B * H, S, 1), np.float32)], axis=2
    )  # [64, 2048, 65]
    # partition-major swizzle for efficient DMA: [64, 128, 16, 65]
    v_sw = np.ascontiguousarray(v_aug.reshape(B * H, NT, 128, 65).transpose(0, 2, 1, 3))
    wp_full = np.ascontiguousarray(W_proj.T)  # [H*Dv, HID]
    if with_mask:
        m_t = np.ascontiguousarray(attention_mask.transpose(0, 1, 3, 2)).reshape(
            B * H, S, S
        )

    in_maps = []
    for c in range(NCORES):
        sl = slice(c * PPC, (c + 1) * PPC)
        m = {
            "kq": kq[sl],
            "vS": v_sw[sl],
            "wp": wp_full[(c % 2) * 512 : (c % 2) * 512 + 512],
        }
        if with_mask:
            m["mT"] = m_t[sl]
        in_maps.append(m)

    global _last_in_maps
    _last_in_maps = in_maps

    nc = _get_nc(with_mask)
    res = run_bass_kernel_spmd(nc, in_maps, core_ids=list(range(NCORES))).results

    # --- gather / unshard ---
    attn_t = np.concatenate([r["attn_t"] for r in res], axis=0)  # [64, S(kj), S(qi)]
    attn = attn_t.reshape(B, H, S, S).transpose(0, 1, 3, 2)
    out = np.stack([res[2 * b]["y"] + res[2 * b + 1]["y"] for b in range(B)])
    return (out, attn)


# revision 8
# speedup vs baseline: 1.0955x; 1.0017x over previous
"""Multi-head attention (B=4, H=16, S=2048, Dqk=Dv=64, HID=1024) on 8 trn2 cores.

Returns (out, attn) matching the reference:
    scores = q @ k^T + mask; attn = softmax(scores); ctx = attn @ v
    out = rearrange(ctx, 'b h s d -> b s (h d)') @ W_proj^T

Sharding: the 64 (b, h) pairs are split 8-per-core (head/data parallel).
Each core computes its pairs' attention probabilities (stored transposed,
[kj, qi], so that both matmuls and the final projection consume natural
layouts) plus a partial projection over its 8 heads; the host sums the
two per-batch partials (the unshard step for the tensor-parallel output).

Device-side layout notes:
  - k^T / q^T are packed per pair into one [64, 4096] block -> single DMA.
  - v gets a ones column appended ([*, 65]); the attn@v matmul then yields
    the softmax denominators in row 64 of PSUM for free.
  - Softmax skips the max-subtraction: |scores| < ~60 for any plausible
    input here, comfortably inside fp32 exp range; -inf masks still work
    (exp(-inf) = 0).
  - matmuls run in fp32r (1 cycle/row at N=512) -> tf32-like rounding,
    ~2e-4 relative error on scores.
"""

import contextlib

import numpy as np

import concourse.bass as bass
import concourse.bacc as bacc
import concourse.mybir as mybir
import concourse.tile as tile
from concourse.bass_utils import run_bass_kernel_spmd

F32 = mybir.dt.float32
F32R = mybir.dt.float32r
EXP = mybir.ActivationFunctionType.Exp
COPY = mybir.ActivationFunctionType.Copy

B, H, S, D, HID = 4, 16, 2048, 64, 1024
NCORES = 8
PPC = B * H // NCORES       # pairs per core = 8
QCHUNK = 512                # query-chunk width (matmul N)
NQC = S // QCHUNK           # 4
NT = S // 128               # 16 key blocks
TH = 2                      # e-tile groups per chunk (8 key blocks each)

_cache: dict = {}
_last_in_maps: list | None = None


def _build(with_mask: bool):
    nc = bacc.Bacc()
    kq_d = nc.declare_dram_parameter("kq", [PPC, 2 * D, 2 * S], F32, isOutput=False)
    v_d = nc.declare_dram_parameter("vS", [PPC, 128, NT, 65], F32, isOutput=False)
    wp_d = nc.declare_dram_parameter("wp", [PPC * D, HID], F32, isOutput=False)
    if with_mask:
        m_d = nc.declare_dram_parameter("mT", [PPC, S, S], F32, isOutput=False)
    attn_d = nc.declare_dram_parameter("attn_t", [PPC, S, S], F32, isOutput=True)
    y_d = nc.declare_dram_parameter("y", [S, HID], F32, isOutput=True)

    with tile.TileContext(nc) as tc:
        with (
            tc.tile_pool(name="kq_p", bufs=2) as kq_p,
            tc.tile_pool(name="v_p", bufs=2) as v_p,
            tc.tile_pool(name="e_p", bufs=3 if with_mask else 4) as e_p,
            tc.tile_pool(name="rb_p", bufs=3) as rb_p,
            tc.tile_pool(name="rc_p", bufs=3) as rc_p,
            tc.tile_pool(name="xt_p", bufs=1) as xt_p,
            tc.tile_pool(name="wp_p", bufs=1) as wp_p,
            tc.tile_pool(name="yo_p", bufs=3) as yo_p,
            tc.tile_pool(name="one_p", bufs=1) as one_p,
            tc.tile_pool(name="ps_p", bufs=4, space="PSUM") as ps_p,
            tc.tile_pool(name="po_p", bufs=2, space="PSUM") as po_p,
            tc.tile_pool(name="pb_p", bufs=2, space="PSUM") as pb_p,
            (
                tc.tile_pool(name="m_p", bufs=2)
                if with_mask
                else contextlib.nullcontext()
            ) as m_p,
        ):
            ones_t = one_p.tile([1, 128], F32, name="ones")
            nc.vector.memset(ones_t, 1.0)

            wp_t = []
            for i in range(4):
                t = wp_p.tile([128, HID], F32R, name=f"wp{i}")
                nc.sync.dma_start(
                    out=t, in_=wp_d[i * 128 : (i + 1) * 128, :].bitcast(F32R)
                )
                wp_t.append(t)

            xT = [xt_p.tile([128, S], F32R, name=f"xT{i}") for i in range(4)]

            for p in range(PPC):
                kq_t = kq_p.tile([2 * D, 2 * S], F32R, name="kq_t")
                nc.sync.dma_start(out=kq_t, in_=kq_d[p].bitcast(F32R))
                v_t = v_p.tile([128, NT, 65], F32R, name="v_t")
                nc.sync.dma_start(out=v_t, in_=v_d[p].bitcast(F32R))

                for qc in range(NQC):
                    q_lo = kq_t[0:D, S + qc * QCHUNK : S + (qc + 1) * QCHUNK]
                    q_hi = kq_t[D:2 * D, S + qc * QCHUNK : S + (qc + 1) * QCHUNK]
                    if with_mask:
                        m_big = []
                        for th in range(TH):
                            mb = m_p.tile([128, 8, QCHUNK], F32, name="m_big")
                            nc.sync.dma_start(
                                out=mb,
                                in_=m_d[
                                    p,
                                    th * 1024 : (th + 1) * 1024,
                                    qc * QCHUNK : (qc + 1) * QCHUNK,
                                ].rearrange("(ts pp) j -> pp ts j", pp=128),
                            )
                            m_big.append(mb)
                    e_big = [
                        e_p.tile([128, 8, QCHUNK], F32R, name="e_big")
                        for _ in range(TH)
                    ]
                    ps_o = po_p.tile([65, QCHUNK], F32, name="ps_o")
                    for tp in range(NT // 2):
                        tA, tB = 2 * tp, 2 * tp + 1
                        ps_A = ps_p.tile([128, QCHUNK], F32, name="ps_s")
                        ps_B = ps_p.tile([128, QCHUNK], F32, name="ps_s")
                        nc.tensor.matmul(
                            ps_A[:],
                            kq_t[0:D, tA * 128 : (tA + 1) * 128],
                            q_lo,
                            start=True,
                            stop=True,
                            tile_position=(0, 0),
                        )
                        nc.tensor.matmul(
                            ps_B[:],
                            kq_t[D : 2 * D, tB * 128 : (tB + 1) * 128],
                            q_hi,
                            start=True,
                            stop=True,
                            tile_position=(64, 0),
                        )
                        for t, ps_s in ((tA, ps_A), (tB, ps_B)):
                            th, ts = divmod(t, 8)
                            if with_mask:
                                nc.vector.tensor_add(
                                    ps_s[:], ps_s[:], m_big[th][:, ts, :]
                                )
                            nc.scalar.activation(e_big[th][:, ts, :], ps_s[:], EXP)
                        for t in (tA, tB):
                            th, ts = divmod(t, 8)
                            nc.tensor.matmul(
                                ps_o[:],
                                v_t[:, t, :],
                                e_big[th][:, ts, :],
                                start=(t == 0),
                                stop=(t == NT - 1),
                            )
                    # reciprocal of the softmax sums (row 64), broadcast to
                    # all 128 partitions via a K=1 fp32 matmul
                    rc = rc_p.tile([1, QCHUNK], F32, name="rc")
                    rc_s = rc_p.tile([1, QCHUNK], F32, name="rc_s")
                    nc.vector.tensor_copy(rc_s[:], ps_o[64:65, :])
                    nc.vector.reciprocal_approx_fast(rc[:], rc_s[:])
                    ps_b = pb_p.tile([128, QCHUNK], F32, name="ps_b")
                    nc.tensor.matmul(ps_b[:], ones_t[:], rc[:], start=True, stop=True)
                    rb = rb_p.tile([128, QCHUNK], F32, name="rb")
                    nc.scalar.activation(rb[:], ps_b[:], COPY)

                    # normalized context (transposed) into the projection stack
                    xi, xr = divmod(p, 2)
                    nc.vector.tensor_mul(
                        xT[xi][xr * 64 : (xr + 1) * 64, qc * QCHUNK : (qc + 1) * QCHUNK],
                        ps_o[0:64, :],
                        rb[0:64, :],
                    )
                    # normalize attention probs in place and store
                    for th in range(TH):
                        for ts in range(8):
                            nc.vector.tensor_mul(
                                e_big[th][:, ts, :], e_big[th][:, ts, :], rb[:]
                            )
                        out_ap = attn_d[
                            p,
                            th * 1024 : (th + 1) * 1024,
                            qc * QCHUNK : (qc + 1) * QCHUNK,
                        ].rearrange("(ts pp) j -> pp ts j", pp=128)
                        nc.sync.dma_start(out=out_ap.bitcast(F32R), in_=e_big[th][:])

            # final projection: y[s, :] = sum_i xT[i][:, s]^T @ wp[i]
            for sc in range(NT):
                y_sb = yo_p.tile([128, HID], F32, name="y_sb")
                for hc in range(2):
                    ps_y = ps_p.tile([128, QCHUNK], F32, name="ps_s")
                    for i in range(4):
                        nc.tensor.matmul(
                            ps_y[:],
                            xT[i][:, sc * 128 : (sc + 1) * 128],
                            wp_t[i][:, hc * QCHUNK : (hc + 1) * QCHUNK],
                            start=(i == 0),
                            stop=(i == 3),
                        )
                    nc.scalar.activation(
                        y_sb[:, hc * QCHUNK : (hc + 1) * QCHUNK], ps_y[:], COPY
                    )
                nc.sync.dma_start(
                    out=y_d[sc * 128 : (sc + 1) * 128, :], in_=y_sb[:]
                )
    nc.compile()
    return nc


def _get_nc(with_mask: bool):
    if with_mask not in _cache:
        _cache[with_mask] = _build(with_mask)
    return _cache[with_mask]


def kernel(q, k, v, attention_mask, W_proj):
    q = np.asarray(q, dtype=np.float32)
    k = np.asarray(k, dtype=np.float32)
    v = np.asarray(v, dtype=np.float32)
    attention_mask = np.asarray(attention_mask, dtype=np.float32)
    W_proj = np.asarray(W_proj, dtype=np.float32)

    with_mask = bool(np.any(attention_mask))

    # --- host-side layout marshalling (sharding) ---
    k_t = np.ascontiguousarray(k.transpose(0, 1, 3, 2)).reshape(B * H, D, S)
    q_t = np.ascontiguousarray(q.transpose(0, 1, 3, 2)).reshape(B * H, D, S)
    kq1 = np.concatenate([k_t, q_t], axis=2)  # [64, 64, 4096]
    kq = np.concatenate([kq1, kq1], axis=1)  # [64, 128, 4096] duplicated halves
    v_flat = v.reshape(B * H, S, D)
    v_aug = np.concatenate(
        [v_flat, np.ones((B * H, S, 1), np.float32)], axis=2
    )  # [64, 2048, 65]
    # partition-major swizzle for efficient DMA: [64, 128, 16, 65]
    v_sw = np.ascontiguousarray(v_aug.reshape(B * H, NT, 128, 65).transpose(0, 2, 1, 3))
    wp_full = np.ascontiguousarray(W_proj.T)  # [H*Dv, HID]
    if with_mask:
        m_t = np.ascontiguousarray(attention_mask.transpose(0, 1, 3, 2)).reshape(
            B * H, S, S
        )

    in_maps = []
    for c in range(NCORES):
        sl = slice(c * PPC, (c + 1) * PPC)
        m = {
            "kq": kq[sl],
            "vS": v_sw[sl],
            "wp": wp_full[(c % 2) * 512 : (c % 2) * 512 + 512],
        }
        if with_mask:
            m["mT"] = m_t[sl]
        in_maps.append(m)

    global _last_in_maps
    _last_in_maps = in_maps

    nc = _get_nc(with_mask)
    res = run_bass_kernel_spmd(nc, in_maps, core_ids=list(range(NCORES))).results

    # --- gather / unshard ---
    attn_t = np.concatenate([r["attn_t"] for r in res], axis=0)  # [64, S(kj), S(qi)]
    attn = attn_t.reshape(B, H, S, S).transpose(0, 1, 3, 2)
    out = np.stack([res[2 * b]["y"] + res[2 * b + 1]["y"] for b in range(B)])
    return (out, attn)


# revision 9
# speedup vs baseline: 1.6438x; 1.5005x over previous
"""Multi-head attention (B=4, H=16, S=2048, Dqk=Dv=64, HID=1024) on 8 trn2 cores.

Returns (out, attn) matching the reference:
    scores = q @ k^T + mask; attn = softmax(scores); ctx = attn @ v
    out = rearrange(ctx, 'b h s d -> b s (h d)') @ W_proj^T

Sharding: the 64 (b, h) pairs are split 8-per-core (head/data parallel).
Each core computes its pairs' attention probabilities (stored transposed,
[kj, qi], so that both matmuls and the final projection consume natural
layouts) plus a partial projection over its 8 heads; the host sums the
two per-batch partials (the unshard step for the tensor-parallel output).

Device-side layout notes:
  - k^T / q^T are packed per pair into one [64, 4096] block -> single DMA.
  - v gets a ones column appended ([*, 65]); the attn@v matmul then yields
    the softmax denominators in row 64 of PSUM for free.
  - Softmax skips the max-subtraction: |scores| < ~60 for any plausible
    input here, comfortably inside fp32 exp range; -inf masks still work
    (exp(-inf) = 0).
  - matmuls run in fp32r (1 cycle/row at N=512) -> tf32-like rounding,
    ~2e-4 relative error on scores.
"""

import contextlib

import numpy as np

import concourse.bass as bass
import concourse.bacc as bacc
import concourse.mybir as mybir
import concourse.tile as tile
from concourse.bass_utils import run_bass_kernel_spmd

F32 = mybir.dt.float32
F32R = mybir.dt.float32r
EXP = mybir.ActivationFunctionType.Exp
COPY = mybir.ActivationFunctionType.Copy

B, H, S, D, HID = 4, 16, 2048, 64, 1024
NCORES = 8
PPC = B * H // NCORES       # pairs per core = 8
QCHUNK = 512                # query-chunk width (matmul N)
NQC = S // QCHUNK           # 4
NT = S // 128               # 16 key blocks
TH = 2                      # e-tile groups per chunk (8 key blocks each)

_cache: dict = {}
_last_in_maps: list | None = None


def _build(with_mask: bool):
    nc = bacc.Bacc()
    kq_d = nc.declare_dram_parameter("kq", [PPC, 2 * D, 2 * S], F32, isOutput=False)
    v_d = nc.declare_dram_parameter("vS", [PPC, 128, NT, 65], F32, isOutput=False)
    wp_d = nc.declare_dram_parameter("wp", [PPC * D, HID], F32, isOutput=False)
    if with_mask:
        m_d = nc.declare_dram_parameter("mT", [PPC, S, S], F32, isOutput=False)
    attn_d = nc.declare_dram_parameter("attn_t", [PPC, S, S], F32, isOutput=True)
    y_d = nc.declare_dram_parameter("y", [S, HID], F32, isOutput=True)

    with tile.TileContext(nc) as tc:
        with (
            tc.tile_pool(name="kq_p", bufs=2) as kq_p,
            tc.tile_pool(name="v_p", bufs=2) as v_p,
            tc.tile_pool(name="e_p", bufs=3 if with_mask else 4) as e_p,
            tc.tile_pool(name="rb_p", bufs=3) as rb_p,
            tc.tile_pool(name="rc_p", bufs=3) as rc_p,
            tc.tile_pool(name="xt_p", bufs=1) as xt_p,
            tc.tile_pool(name="wp_p", bufs=1) as wp_p,
            tc.tile_pool(name="yo_p", bufs=3) as yo_p,
            tc.tile_pool(name="one_p", bufs=1) as one_p,
            tc.tile_pool(name="ps_p", bufs=4, space="PSUM") as ps_p,
            tc.tile_pool(name="po_p", bufs=2, space="PSUM") as po_p,
            tc.tile_pool(name="pb_p", bufs=2, space="PSUM") as pb_p,
            (
                tc.tile_pool(name="m_p", bufs=2)
                if with_mask
                else contextlib.nullcontext()
            ) as m_p,
        ):
            ones_t = one_p.tile([1, 128], F32, name="ones")
            nc.vector.memset(ones_t, 1.0)

            wp_t = []
            for i in range(4):
                t = wp_p.tile([128, HID], F32R, name=f"wp{i}")
                nc.sync.dma_start(
                    out=t, in_=wp_d[i * 128 : (i + 1) * 128, :].bitcast(F32R)
                )
                wp_t.append(t)

            xT = [xt_p.tile([128, S], F32R, name=f"xT{i}") for i in range(4)]

            for p in range(PPC):
                kq_t = kq_p.tile([2 * D, 2 * S], F32R, name="kq_t")
                nc.sync.dma_start(out=kq_t, in_=kq_d[p].bitcast(F32R))
                v_t = v_p.tile([128, NT, 65], F32R, name="v_t")
                nc.sync.dma_start(out=v_t, in_=v_d[p].bitcast(F32R))

                for qc in range(NQC):
                    q_sl = kq_t[:, S + qc * QCHUNK : S + (qc + 1) * QCHUNK]
                    if with_mask:
                        m_big = []
                        for th in range(TH):
                            mb = m_p.tile([128, 8, QCHUNK], F32, name="m_big")
                            nc.sync.dma_start(
                                out=mb,
                                in_=m_d[
                                    p,
                                    th * 1024 : (th + 1) * 1024,
                                    qc * QCHUNK : (qc + 1) * QCHUNK,
                                ].rearrange("(ts pp) j -> pp ts j", pp=128),
                            )
                            m_big.append(mb)
                    e_big = [
                        e_p.tile([128, 8, QCHUNK], F32R, name="e_big")
                        for _ in range(TH)
                    ]
                    ps_o = po_p.tile([65, QCHUNK], F32, name="ps_o")
                    for t in range(NT):
                        th, ts = divmod(t, 8)
                        ps_s = ps_p.tile([128, QCHUNK], F32, name="ps_s")
                        # K padded to 128 (rows 64-127 zero) so every matmul
                        # is full-row: keeps the PE activity monitor warm.
                        nc.tensor.matmul(
                            ps_s[:],
                            kq_t[:, t * 128 : (t + 1) * 128],
                            q_sl,
                            start=True,
                            stop=True,
                        )
                        if with_mask:
                            nc.vector.tensor_add(
                                ps_s[:], ps_s[:], m_big[th][:, ts, :]
                            )
                        nc.scalar.activation(e_big[th][:, ts, :], ps_s[:], EXP)
                        nc.tensor.matmul(
                            ps_o[:],
                            v_t[:, t, :],
                            e_big[th][:, ts, :],
                            start=(t == 0),
                            stop=(t == NT - 1),
                        )
                    # reciprocal of the softmax sums (row 64), broadcast to
                    # all 128 partitions via a K=1 fp32 matmul
                    rc = rc_p.tile([1, QCHUNK], F32, name="rc")
                    rc_s = rc_p.tile([1, QCHUNK], F32, name="rc_s")
                    nc.vector.tensor_copy(rc_s[:], ps_o[64:65, :])
                    nc.vector.reciprocal_approx_fast(rc[:], rc_s[:])
                    ps_b = pb_p.tile([128, QCHUNK], F32, name="ps_b")
                    nc.tensor.matmul(ps_b[:], ones_t[:], rc[:], start=True, stop=True)
                    rb = rb_p.tile([128, QCHUNK], F32, name="rb")
                    nc.scalar.activation(rb[:], ps_b[:], COPY)

                    # normalized context (transposed) into the projection stack
                    xi, xr = divmod(p, 2)
                    nc.vector.tensor_mul(
                        xT[xi][xr * 64 : (xr + 1) * 64, qc * QCHUNK : (qc + 1) * QCHUNK],
                        ps_o[0:64, :],
                        rb[0:64, :],
                    )
                    # normalize attention probs in place and store
                    for th in range(TH):
                        for ts in range(8):
                            nc.vector.tensor_mul(
                                e_big[th][:, ts, :], e_big[th][:, ts, :], rb[:]
                            )
                        out_ap = attn_d[
                            p,
                            th * 1024 : (th + 1) * 1024,
                            qc * QCHUNK : (qc + 1) * QCHUNK,
                        ].rearrange("(ts pp) j -> pp ts j", pp=128)
                        nc.sync.dma_start(out=out_ap.bitcast(F32R), in_=e_big[th][:])

            # final projection: y[s, :] = sum_i xT[i][:, s]^T @ wp[i]
            for sc in range(NT):
                y_sb = yo_p.tile([128, HID], F32, name="y_sb")
                for hc in range(2):
                    ps_y = ps_p.tile([128, QCHUNK], F32, name="ps_s")
                    for i in range(4):
                        nc.tensor.matmul(
                            ps_y[:],
                            xT[i][:, sc * 128 : (sc + 1) * 128],
                            wp_t[i][:, hc * QCHUNK : (hc + 1) * QCHUNK],
                            start=(i == 0),
                            stop=(i == 3),
                        )
                    nc.scalar.activation(
                        y_sb[:, hc * QCHUNK : (hc + 1) * QCHUNK], ps_y[:], COPY
                    )
                nc.sync.dma_start(
                    out=y_d[sc * 128 : (sc + 1) * 128, :], in_=y_sb[:]
                )
    nc.compile()
    return nc


def _get_nc(with_mask: bool):
    if with_mask not in _cache:
        _cache[with_mask] = _build(with_mask)
    return _cache[with_mask]


def kernel(q, k, v, attention_mask, W_proj):
    q = np.asarray(q, dtype=np.float32)
    k = np.asarray(k, dtype=np.float32)
    v = np.asarray(v, dtype=np.float32)
    attention_mask = np.asarray(attention_mask, dtype=np.float32)
    W_proj = np.asarray(W_proj, dtype=np.float32)

    with_mask = bool(np.any(attention_mask))

    # --- host-side layout marshalling (sharding) ---
    k_t = np.ascontiguousarray(k.transpose(0, 1, 3, 2)).reshape(B * H, D, S)
    q_t = np.ascontiguousarray(q.transpose(0, 1, 3, 2)).reshape(B * H, D, S)
    kq1 = np.concatenate([k_t, q_t], axis=2)  # [64, 64, 4096]
    # zero-pad the contraction dim to 128 -> full-row matmuls (PE stays warm)
    kq = np.concatenate([kq1, np.zeros_like(kq1)], axis=1)  # [64, 128, 4096]
    v_flat = v.reshape(B * H, S, D)
    v_aug = np.concatenate(
        [v_flat, np.ones((B * H, S, 1), np.float32)], axis=2
    )  # [64, 2048, 65]
    # partition-major swizzle for efficient DMA: [64, 128, 16, 65]
    v_sw = np.ascontiguousarray(v_aug.reshape(B * H, NT, 128, 65).transpose(0, 2, 1, 3))
    wp_full = np.ascontiguousarray(W_proj.T)  # [H*Dv, HID]
    if with_mask:
        m_t = np.ascontiguousarray(attention_mask.transpose(0, 1, 3, 2)).reshape(
            B * H, S, S
        )

    in_maps = []
    for c in range(NCORES):
        sl = slice(c * PPC, (c + 1) * PPC)
        m = {
            "kq": kq[sl],
            "vS": v_sw[sl],
            "wp": wp_full[(c % 2) * 512 : (c % 2) * 512 + 512],
        }
        if with_mask:
            m["mT"] = m_t[sl]
        in_maps.append(m)

    global _last_in_maps
    _last_in_maps = in_maps

    nc = _get_nc(with_mask)
    res = run_bass_kernel_spmd(nc, in_maps, core_ids=list(range(NCORES))).results

    # --- gather / unshard ---
    attn_t = np.concatenate([r["attn_t"] for r in res], axis=0)  # [64, S(kj), S(qi)]
    attn = attn_t.reshape(B, H, S, S).transpose(0, 1, 3, 2)
    out = np.stack([res[2 * b]["y"] + res[2 * b + 1]["y"] for b in range(B)])
    return (out, attn)
